# revision 11
# baseline (speedup 1.0000x reference)
"""Trainium2 Bass kernel for nn_Att_21973052686618.

Dense transformer block: QKV projection (512->10, 2 heads x d_head=5),
sin-activated causal attention, softmax, 2-layer MLP head (10->20->512).

Sharding: data-parallel over batch B=16 across 8 NeuronCores (2 batches/core).

Per-core pipeline:
  - x is host-cast to bf16 and DMA-transposed from DRAM into xT tiles.
  - PE: QKV projections (bf16), per-k-chunk transposed scores (float32r),
    column-tiled attention*V (bf16, 4 q-chunks packed into one PSUM bank,
    with a fused ones-column producing the softmax denominator).
  - ACT: sin(score/sqrt(5)) from PSUM (the only engine with sin), and the
    MLP's leaky_relu -- both live in the same activation-table set, so the
    table is loaded exactly once.
  - DVE: a custom 8-slice op computing exp(u) ~ min(1 + c1 u + c2 u^2, m)^4
    (valid after sin since u in [-1,1]; the min() also clamps the rare
    garbage outputs sin emits for |arg| > ~8.7), with a variant that fuses
    the causal-diagonal mask multiply; plus PSUM evictions and reciprocal.
"""

import math
import sys

import numpy as np

sys.path.insert(0, "/opt/trn_rl_repo")

import ml_dtypes  # noqa: E402

import concourse.bass as bass  # noqa: E402,F401
import concourse.tile as tile  # noqa: E402
import concourse.dve_ops as dve_ops  # noqa: E402
from concourse import bacc, mybir  # noqa: E402
from concourse.bass_utils import run_bass_kernel_spmd  # noqa: E402
from concourse.dve_spec import (  # noqa: E402
    C0,
    C1,
    C2,
    InpSel,
    Leaf,
    Spec,
    Src0,
    Src1,
    lower,
    minn,
    sq,
)
from concourse.dve_uop import DveOpSpec  # noqa: E402
from contextlib import ExitStack  # noqa: E402

F32 = mybir.dt.float32
F32R = mybir.dt.float32r
BF16 = mybir.dt.bfloat16

B, N, DIM_IN, DIM_OUT = 16, 2048, 512, 512
D_K, N_HEADS, D_HEAD = 10, 2, 5
NCORES = 8
BPC = B // NCORES  # batches per core
NKC = N // 128  # 16 k-chunks of 128 rows
NQC = N // 512  # 4 q-chunks of 512
INV_SQRT_DH = 1.0 / math.sqrt(D_HEAD)
import os
SCORES_DT_ENV = os.environ.get("SCORES_DT", "f32r")
SCORES_DT = None  # resolved in _build_program

# --- exp(u) ~ A * min(1 + c1 u + c2 u^2, CLAMP)^4 for u in [-1, 1] -------
# Coefficients: minimax fit of e^{u/4} by a quadratic, rescaled so the
# constant term is exactly 1 (softmax is invariant to the overall scale A).
# Max relative error of the 4th power vs e^u is ~2.6e-3; the quadratic has
# no real roots so the base is always positive, and CLAMP bounds the output
# for wildly out-of-range u.
EXP4_C1 = 0.2494760427419436
EXP4_C2 = 0.031046627166491383
EXP4_CLAMP = 1.2840254166877414  # e^{1/4}

_ONE = Leaf(InpSel.ONE_F32)


def _register_exp_ops():
    """Register the custom DVE exp ops into the dve_ops registry (idempotent)."""
    if any(op.name == "EXP4_ANT" for op in dve_ops.OPS):
        ops = {op.name: op for op in dve_ops.OPS}
        return ops["EXP4_ANT"], ops["EXP4M_ANT"]

    def make_op(name, body, ref):
        spec = Spec(body=body, reference=ref)
        shas = {}
        for ver in ("v3", "v4"):
            uops = lower(spec, ver=ver)
            shas[ver] = DveOpSpec(name=name, opcode=1, uops=uops, rd1_en=False).sha(ver)
        op = dve_ops.DveOp(name, spec, subdim=False, uops_sha=shas)
        dve_ops.OPS.append(op)
        dve_ops.CUSTOM_DVE_SPECS[op.name] = op.spec
        dve_ops._SUB_OPCODE_FOR_NAME[op.name] = (
            dve_ops._CUSTOM_DVE_ROW_BASE + len(dve_ops.OPS) - 1
        )
        return op

    def _ref_exp4(in0, in1, s0, s1, imm2):
        x = in0.astype(np.float32)
        return np.minimum(1.0 + x * (s0 + s1 * x), imm2) ** 4

    def _ref_exp4m(in0, in1, s0, s1, imm2):
        return _ref_exp4(in0, in1, s0, s1, imm2) * in1.astype(np.float32)

    exp4 = make_op(
        "EXP4_ANT",
        sq(sq(minn(_ONE + Src0 * (C0 + C1 * Src0), C2))),
        _ref_exp4,
    )
    exp4m = make_op(
        "EXP4M_ANT",
        sq(sq(minn(_ONE + Src0 * (C0 + C1 * Src0), C2))) * Src1,
        _ref_exp4m,
    )
    return exp4, exp4m


def _build_program(repeat=1):
    """Build (once) the single-core Bass program run SPMD on all 8 cores."""
    global SCORES_DT
    SCORES_DT = F32 if SCORES_DT_ENV == "f32" else F32R
    EXP4, EXP4M = _register_exp_ops()
    nc = bacc.Bacc("TRN2", target_bir_lowering=False, debug=False, num_devices=NCORES)

    # ---- DRAM parameters (per-core inputs / outputs) ----
    xbf = nc.dram_tensor("xbf", [BPC, N, DIM_IN], BF16, kind="ExternalInput").ap()
    out = nc.dram_tensor("out", [BPC, N, DIM_OUT], F32, kind="ExternalOutput").ap()
    # weights / constants (shared values, same on every core)
    wqk_d = nc.dram_tensor("wqk", [DIM_IN, 2 * D_K], BF16, kind="ExternalInput").ap()
    wvp_d = nc.dram_tensor("wvp", [DIM_IN, 11], BF16, kind="ExternalInput").ap()
    bqk_d = nc.dram_tensor("bqk", [2 * D_K, 1], F32, kind="ExternalInput").ap()
    bvp_d = nc.dram_tensor("bvp", [128, 11], F32, kind="ExternalInput").ap()
    mask_d = nc.dram_tensor("mask128", [128, 128], BF16, kind="ExternalInput").ap()
    diag_d = nc.dram_tensor("diagc", [128, 128], BF16, kind="ExternalInput").ap()
    wm1_d = nc.dram_tensor("wm1", [D_K, 20], F32, kind="ExternalInput").ap()
    bm1_d = nc.dram_tensor("bm1", [20, 1], F32, kind="ExternalInput").ap()
    wm2p_d = nc.dram_tensor("wm2p", [21, DIM_OUT], F32, kind="ExternalInput").ap()
    ones_d = nc.dram_tensor("onesrow", [1, N], F32, kind="ExternalInput").ap()
    sel_d = nc.dram_tensor("sel4", [2 * NQC, 128], F32, kind="ExternalInput").ap()

    with tile.TileContext(nc) as tc, ExitStack() as ctx:
        # ---- pools ----
        consts = ctx.enter_context(tc.tile_pool(name="consts", bufs=1))
        xt_pool = ctx.enter_context(tc.tile_pool(name="xt", bufs=4))
        qkt_pool = ctx.enter_context(tc.tile_pool(name="qkt", bufs=1))
        stage_pool = ctx.enter_context(tc.tile_pool(name="stage", bufs=3))
        v2_pool = ctx.enter_context(tc.tile_pool(name="v2", bufs=2))
        u_pool = ctx.enter_context(tc.tile_pool(name="u", bufs=3))
        a1_pool = ctx.enter_context(tc.tile_pool(name="a1", bufs=3))
        p_pool = ctx.enter_context(tc.tile_pool(name="pp", bufs=4))
        valsb_pool = ctx.enter_context(tc.tile_pool(name="valsb", bufs=2))
        sums_pool = ctx.enter_context(tc.tile_pool(name="sums", bufs=1))
        recip_pool = ctx.enter_context(tc.tile_pool(name="recip", bufs=1))
        normed_pool = ctx.enter_context(tc.tile_pool(name="normed", bufs=2))
        concat_pool = ctx.enter_context(tc.tile_pool(name="concat", bufs=2))
        h1_pool = ctx.enter_context(tc.tile_pool(name="h1", bufs=2))
        outs_pool = ctx.enter_context(tc.tile_pool(name="outs", bufs=3))
        # PSUM: exactly 8 banks total
        psA = ctx.enter_context(tc.tile_pool(name="psA", bufs=2, space="PSUM"))  # 2x2
        psV = ctx.enter_context(tc.tile_pool(name="psV", bufs=2, space="PSUM"))  # 2x1
        psB = ctx.enter_context(tc.tile_pool(name="psB", bufs=2, space="PSUM"))  # 2x1

        # ---- load constants ----
        wqk = [consts.tile([128, 2 * D_K], BF16, tag=f"wqk{f}", name=f"wqk{f}") for f in range(4)]
        wvp = [consts.tile([128, 11], BF16, tag=f"wvp{f}", name=f"wvp{f}") for f in range(4)]
        for f in range(4):
            nc.sync.dma_start(wqk[f][:], wqk_d[128 * f : 128 * (f + 1), :])
            nc.sync.dma_start(wvp[f][:], wvp_d[128 * f : 128 * (f + 1), :])
        bqk = consts.tile([2 * D_K, 1], F32, tag="bqk")
        nc.sync.dma_start(bqk[:], bqk_d[:])
        bvp = consts.tile([128, 11], F32, tag="bvp")
        nc.sync.dma_start(bvp[:], bvp_d[:])
        mask = consts.tile([128, 128], BF16, tag="mask")
        nc.sync.dma_start(mask[:], mask_d[:])
        diagc = consts.tile([128, 128], BF16, tag="diagc")
        nc.sync.dma_start(diagc[:], diag_d[:])
        b192 = consts.tile([128, 1], F32, tag="b192")
        nc.vector.memset(b192[:], 192.0)
        bsin = consts.tile([128, 1], F32, tag="bsin")
        # cancels c_hi*192 (c_hi = bf16-rounded -2pi*sqrt5 used in the inject)
        c_hi_ = float(np.float32(ml_dtypes.bfloat16(-2.0 * math.pi * math.sqrt(5.0))))
        nc.vector.memset(bsin[:], float(-c_hi_ * 192.0 / math.sqrt(5.0)))
        bm1 = consts.tile([20, 1], F32, tag="bm1")
        nc.sync.dma_start(bm1[:], bm1_d[:])
        wm1f = consts.tile([D_K, 20], F32, tag="wm1f")
        nc.sync.dma_start(wm1f[:], wm1_d[:])
        wm1 = consts.tile([D_K, 20], F32R, tag="wm1")
        nc.vector.tensor_copy(wm1[:], wm1f[:])
        wm2f = consts.tile([21, DIM_OUT], F32, tag="wm2f")
        nc.sync.dma_start(wm2f[:], wm2p_d[:])
        wm2 = consts.tile([21, DIM_OUT], F32R, tag="wm2")
        nc.vector.tensor_copy(wm2[:], wm2f[:])
        sel4 = []
        for h in range(2):
            sf = consts.tile([NQC, 128], F32, tag=f"self{h}", name=f"self{h}")
            nc.sync.dma_start(sf[:], sel_d[NQC * h : NQC * (h + 1), :])
            sr = consts.tile([NQC, 128], F32R, tag=f"sel4{h}", name=f"sel4{h}")
            nc.vector.tensor_copy(sr[:], sf[:])
            sel4.append(sr)

        def exp_call(p_ap, u_ap, masked):
            op = EXP4M if masked else EXP4
            kw = dict(s0=EXP4_C1, s1=EXP4_C2, imm2=EXP4_CLAMP)
            if masked:
                nc.vector._custom_dve(op, out=p_ap, in0=u_ap, in1=mask[:], **kw)
            else:
                nc.vector._custom_dve(op, out=p_ap, in0=u_ap, **kw)

        for _rep in range(repeat):
         for b in range(BPC):
            # ---- load xT (transposing DMA straight from DRAM) ----
            xT = []
            for f in range(4):
                t = xt_pool.tile([128, N], BF16, tag="xt", name=f"xt_b{b}_{f}", bufs=4)
                nc.sync.dma_start(
                    t[:], xbf[b, :, 128 * f : 128 * (f + 1)], transpose=True
                )
                xT.append(t)

            # ---- qk projection into qkT5 [5, 4N] (all on partitions 0-4) ----
            # free blocks: [0:N)=q_h0, [N:2N)=k_h0, [2N:3N)=q_h1, [3N:4N)=k_h1
            # wqk columns are ordered to match: q_h0, k_h0, q_h1, k_h1
            qkT5 = qkt_pool.tile([5, 4 * N], SCORES_DT, tag="qkT5")
            for rc in range(NQC):
                pq = psA.tile([128, 1024], F32, tag="psA")
                for f in range(4):
                    nc.tensor.matmul(
                        pq[0 : 2 * D_K, 0:512],
                        wqk[f][:],
                        xT[f][:, 512 * rc : 512 * (rc + 1)],
                        start=(f == 0),
                        stop=(f == 3),
                    )
                # evict + bias (per-partition scalar) + round to f32r
                stg = stage_pool.tile([2 * D_K, 512], SCORES_DT, tag="stage")
                nc.vector.tensor_scalar_add(stg[:], pq[0 : 2 * D_K, 0:512], bqk[:])
                # partition remap (rows 5g:5g+5 -> block g) via idle SWDGE DMAs
                for g in range(4):
                    nc.gpsimd.dma_start(
                        qkT5[0:5, N * g + 512 * rc : N * g + 512 * (rc + 1)],
                        stg[5 * g : 5 * (g + 1), :],
                    )

            # ---- v projection into v' layout [128, 11 per k-chunk] bf16 ----
            # columns 11c+0..4 = head0 v dims, 11c+5 = ones, 11c+6..10 = head1
            v2 = v2_pool.tile([128, 11 * NKC], BF16, tag="v2")
            for c in range(NKC):
                pv = psA.tile([128, 1024], F32, tag="psA")
                for f in range(4):
                    nc.tensor.matmul(
                        pv[:, 0:11],
                        xT[f][:, 128 * c : 128 * (c + 1)],
                        wvp[f][:],
                        start=(f == 0),
                        stop=(f == 3),
                    )
                nc.vector.tensor_tensor(
                    v2[:, 11 * c : 11 * (c + 1)],
                    pv[:, 0:11],
                    bvp[:],
                    mybir.AluOpType.add,
                )

            # ---- attention per head ----
            concatT = concat_pool.tile([D_K, N], F32R, tag="concatT")
            for h in range(2):
                vals = psV.tile([128, 512], F32, tag="psV")
                qT = qkT5[0:5, 2 * N * h : 2 * N * h + N]
                kT = qkT5[0:5, 2 * N * h + N : 2 * N * h + 2 * N]
                for c in range(NKC):
                    q0 = 128 * c
                    # rounds of up to 1024 q-columns, aligned to 1024 grid
                    r0 = q0 // 1024
                    for r in range(r0, N // 1024):
                        qs = max(q0, 1024 * r)
                        qe = 1024 * (r + 1)
                        seg = qe - qs
                        sc = psA.tile([128, 1024], F32, tag="psA")
                        # scores^T chunk [128 k, seg q] in <=512 column strips
                        off = 0
                        while off < seg:
                            w = min(512, seg - off)
                            nc.tensor.matmul(
                                sc[:, off : off + w],
                                kT[:, 128 * c : 128 * (c + 1)],
                                qT[:, qs + off : qs + off + w],
                                start=True,
                                stop=False,
                                skip_group_check=True,
                            )
                            off += w
                        # range reduction: the HW sin diverges for |arg|>~4,
                        # so compute k = round(s/(2pi*sqrt5)) via the bf16
                        # +192 rounding trick and inject -2pi*sqrt5*k back
                        # into the scores PSUM with a diagonal matmul. The
                        # +192 offset is cancelled by the sin bias below.
                        a1 = a1_pool.tile([128, 1024], BF16, tag="a1")
                        nc.scalar.activation(
                            a1[:, 0:seg],
                            sc[:, 0:seg],
                            mybir.ActivationFunctionType.Identity,
                            bias=b192[:],
                            scale=float(1.0 / (2.0 * math.pi * math.sqrt(5.0))),
                        )
                        off = 0
                        while off < seg:
                            w = min(512, seg - off)
                            nc.tensor.matmul(
                                sc[:, off : off + w],
                                diagc[:],
                                a1[:, off : off + w],
                                start=False,
                                stop=True,
                                skip_group_check=True,
                            )
                            off += w
                        # sin((s - 2pi*sqrt5*(k+192))/sqrt5 + 2pi*192) = sin(wrapped phi)
                        u = u_pool.tile([128, 1024], F32, tag="u")
                        nc.scalar.activation(
                            u[:, 0:seg],
                            sc[:, 0:seg],
                            mybir.ActivationFunctionType.Sin,
                            scale=INV_SQRT_DH,
                            bias=bsin[:],
                        )
                        # exp via custom DVE op; first 128 cols of the
                        # diagonal round get the fused causal mask
                        pt = p_pool.tile([128, 1024], BF16, tag="pp")
                        if qs == q0:
                            exp_call(pt[:, 0:128], u[:, 0:128], masked=True)
                            if seg > 128:
                                exp_call(pt[:, 128:seg], u[:, 128:seg], masked=False)
                        else:
                            exp_call(pt[:, 0:seg], u[:, 0:seg], masked=False)
                        # AV: accumulate vals[32j:32j+6, :] over k-chunks
                        voff = 11 * c + (5 if h == 1 else 0)
                        off = 0
                        while off < seg:
                            qg = qs + off  # global q of this strip start
                            j = qg // 512
                            w = min(512, 512 * (j + 1) - qg)
                            w = min(w, seg - off)
                            col = qg - 512 * j
                            nc.tensor.matmul(
                                vals[32 * j : 32 * j + 6, col : col + w],
                                v2[:, voff : voff + 6],
                                pt[:, off : off + w],
                                start=(c == 0),
                                stop=(c == 4 * j + 3),
                                tile_position=(0, 32 * j),
                                skip_group_check=True,
                            )
                            off += w
                # ---- normalize: concat_h = vals_h / sums_h ----
                valsb = valsb_pool.tile([128, 512], F32, tag="valsb")
                nc.vector.tensor_copy(valsb[:], vals[:])
                # gather per-q-chunk sum rows -> packed [4, 512]
                sums = sums_pool.tile([NQC, 512], F32, tag="sums")
                srow = 5 if h == 0 else 0  # ones-col position within lhsT
                for j in range(NQC):
                    nc.gpsimd.dma_start(
                        sums[j : j + 1, :], valsb[32 * j + srow : 32 * j + srow + 1, :]
                    )
                recips_f = recip_pool.tile([NQC, 512], F32, tag="recipf")
                nc.vector.reciprocal_approx_fast(recips_f[:], sums[:])
                recips = recip_pool.tile([NQC, 512], F32R, tag="recip")
                nc.vector.tensor_copy(recips[:], recips_f[:])
                # broadcast recips back to the vals partition layout via PE
                rb = psB.tile([128, 512], F32, tag="psB")
                nc.tensor.matmul(
                    rb[:], sel4[h][:], recips[:], start=True, stop=True
                )
                normed = normed_pool.tile([128, 512], F32R, tag="normed")
                nc.vector.tensor_tensor(
                    normed[:], valsb[:], rb[:], mybir.AluOpType.mult
                )
                # scatter normalized dims into concatT rows [5h:5h+5]
                d0 = 0 if h == 0 else 1
                for j in range(NQC):
                    nc.gpsimd.dma_start(
                        concatT[5 * h : 5 * h + 5, 512 * j : 512 * (j + 1)],
                        normed[32 * j + d0 : 32 * j + d0 + 5, :],
                    )

            # ---- MLP: h1T = lrelu(Wm1^T @ concatT + bm1) ----
            h1T = h1_pool.tile([21, N], F32R, tag="h1T")
            nc.gpsimd.dma_start(h1T[20:21, :], ones_d[:].bitcast(F32R))
            for rc in range(NQC):
                hp = psB.tile([128, 512], F32, tag="psB")
                nc.tensor.matmul(
                    hp[0:20, :],
                    wm1[:],
                    concatT[:, 512 * rc : 512 * (rc + 1)],
                    start=True,
                    stop=True,
                )
                nc.scalar.activation(
                    h1T[0:20, 512 * rc : 512 * (rc + 1)],
                    hp[0:20, :],
                    mybir.ActivationFunctionType.Lrelu,
                    bias=bm1[:],
                    scale=1.0,
                    alpha=0.01,
                )

            # ---- final: out = h1T^T @ Wm2' (bias folded via ones row) ----
            for rc in range(NKC):
                op_ = psB.tile([128, 512], F32, tag="psB")
                nc.tensor.matmul(
                    op_[:],
                    h1T[:, 128 * rc : 128 * (rc + 1)],
                    wm2[:],
                    start=True,
                    stop=True,
                )
                so = outs_pool.tile([128, 512], F32, tag="outs")
                if rc % 2 == 0:
                    nc.vector.tensor_copy(so[:], op_[:])
                else:
                    nc.scalar.copy(so[:], op_[:])
                nc.sync.dma_start(out[b, 128 * rc : 128 * (rc + 1), :], so[:])

    nc.compile()
    return nc


_NC_CACHE = None


def _get_program():
    global _NC_CACHE
    if _NC_CACHE is None:
        _NC_CACHE = _build_program()
    return _NC_CACHE


def kernel(x, Wq, bq, Wk, bk, Wv, bv, Wm1, bm1, Wm2, bm2, _want_trace=False):
    nc = _get_program()

    x = np.asarray(x, np.float32)
    xbf = x.astype(ml_dtypes.bfloat16)

    Wq_, Wk_ = np.asarray(Wq, np.float32), np.asarray(Wk, np.float32)
    wqk = np.concatenate(
        [Wq_[:, 0:5], Wk_[:, 0:5], Wq_[:, 5:10], Wk_[:, 5:10]], axis=1
    ).astype(ml_dtypes.bfloat16)  # [512, 20] blocks q_h0,k_h0,q_h1,k_h1
    wvp = np.zeros((DIM_IN, 11), np.float32)
    wvp[:, 0:5] = np.asarray(Wv)[:, 0:5]
    wvp[:, 6:11] = np.asarray(Wv)[:, 5:10]
    wvp = wvp.astype(ml_dtypes.bfloat16)
    bq_, bk_ = np.asarray(bq, np.float32), np.asarray(bk, np.float32)
    bqk = np.concatenate([bq_[0:5], bk_[0:5], bq_[5:10], bk_[5:10]])[:, None].astype(
        np.float32
    )
    bvp = np.zeros((128, 11), np.float32)
    bvp[:, 0:5] = np.asarray(bv)[0:5]
    bvp[:, 5] = 1.0
    bvp[:, 6:11] = np.asarray(bv)[5:10]
    mask128 = np.triu(np.ones((128, 128), np.float32)).astype(ml_dtypes.bfloat16)
    c_wrap = -2.0 * np.pi * np.sqrt(5.0)
    c_hi = float(np.float32(ml_dtypes.bfloat16(c_wrap)))
    diagc = (c_hi * np.eye(128, dtype=np.float32)).astype(ml_dtypes.bfloat16)
    wm1 = np.asarray(Wm1, np.float32)
    bm1c = np.asarray(bm1, np.float32)[:, None]
    wm2p = np.concatenate(
        [np.asarray(Wm2, np.float32), np.asarray(bm2, np.float32)[None, :]], axis=0
    )  # [21, 512]
    onesrow = np.ones((1, N), np.float32)
    sel4 = np.zeros((2 * NQC, 128), np.float32)
    for j in range(NQC):
        sel4[j, 32 * j : 32 * j + 5] = 1.0  # head0: dims at 32j+0..4
        sel4[NQC + j, 32 * j + 1 : 32 * j + 6] = 1.0  # head1: dims at 32j+1..5
    shared = dict(
        wqk=wqk,
        wvp=wvp,
        bqk=bqk,
        bvp=bvp,
        mask128=mask128,
        diagc=diagc,
        wm1=wm1,
        bm1=bm1c,
        wm2p=wm2p,
        onesrow=onesrow,
        sel4=sel4,
    )
    in_maps = []
    for core in range(NCORES):
        m = dict(shared)
        m["xbf"] = xbf[BPC * core : BPC * (core + 1)]
        in_maps.append(m)

    res = run_bass_kernel_spmd(
        nc, in_maps, list(range(NCORES)), trace=_want_trace
    )
    outp = np.concatenate([res.results[i]["out"] for i in range(NCORES)], axis=0)
    if _want_trace:
        return outp, res
    return outp
